# revision 51
# baseline (speedup 1.0000x reference)
"""DeepSeek-V3.1 decoder block on 8 Trainium2 NeuronCores (fp8 DoubleRow).

Sharding: core c -> batch b=c//4, position p=c%4; each core owns 4 query
chunks of 128 tokens (chunks {p, 7-p, 8+p, 15-p}) which balances causal
attention. KV projections for the full sequence are replicated within a
batch group (no collectives).

All large GEMMs run as fp8e4m3 DoubleRow matmuls. The attention path
(q_a/q_b/kv_a/kv_b/o_proj) runs single-term fp8 (w_hi*x_hi, 0.25x the
bf16 PE cost); its absolute error contribution is small because the
attention output is a small fraction of the residual stream. The FFN
runs a 2-term split keeping activation precision (w_hi*x_hi + w_hi*x_lo,
0.5x bf16 PE cost) which also halves FFN weight DMA. Attention scores
pack (k_nope | k_rope) into the two DoubleRow slots; PV packs adjacent
key tiles; softmax denominators accumulate on the PE via ones-DoubleRow
matmuls. All scales are powers of two folded into eviction scales /
rstat factors.
"""

import sys

sys.path.insert(0, "/opt/trn_rl_repo")

import numpy as np
import ml_dtypes

import concourse.bass as bass
import concourse.mybir as mybir
from concourse import bacc
from concourse.tile import TileContext
from concourse.bass_utils import run_bass_kernel_spmd

B, S, D = 2, 2048, 2048
H, NOPE, ROPE, VH = 16, 128, 64, 128
QL, KVL, FF = 1024, 512, 8192
BASE, EPS = 10000.0, 1e-6
P = 128
T = 512             # query tokens per core
NCH = S // P        # 16 key tiles per batch
NQ = T // P         # 4 q-slots per core
DT = D // P         # 16
DP = DT // 2        # 8 contraction pairs over D
QLT = QL // P       # 8
KVT = KVL // P      # 4
FFT = FF // P       # 64
SM = float(1.0 / np.sqrt(np.float32(NOPE + ROPE)))

# power-of-two fp8 scales
SW = 256.0          # all weights
SX = 8.0            # hidden/x activations
SQ = 8.0            # q_a output
SC = 8.0            # ckv (normalized latent)
SG = 4.0            # gate*up product
SV = 32.0           # v and attn output
SKN = 32.0          # k-side score operands (k_nope, k_rope)
SQN = 256.0         # q-side score operands (carry SM)

F32 = mybir.dt.float32
BF16 = mybir.dt.bfloat16
FP8 = mybir.dt.float8e4
AF = mybir.ActivationFunctionType
ALU = mybir.AluOpType
DR = mybir.MatmulPerfMode.DoubleRow


def chunks_for_pos(p):
    return [p, 7 - p, 8 + p, 15 - p]


# ------------------------------------------------------------------ device

def _rstat(nc, pool, ps_ap, inv_n, scale_sq, out_bcast, chans, width):
    """out_bcast[:chans,:width] = sqrt(scale_sq / (ps*inv_n + eps))."""
    for n in range(width // 512):
        sl = slice(n * 512, (n + 1) * 512)
        ms = pool.tile([1, 512], F32, tag="rs_ms")
        nc.scalar.activation(ms[0:1, :], ps_ap[0:1, sl], AF.Copy, scale=inv_n)
        nc.vector.tensor_scalar_add(ms[0:1, :], ms[0:1, :], EPS)
        inv = pool.tile([1, 512], F32, tag="rs_inv")
        nc.vector.reciprocal(inv[0:1, :], ms[0:1, :])
        r = pool.tile([1, 512], F32, tag="rs_r")
        nc.scalar.activation(r[0:1, :], inv[0:1, :], AF.Sqrt, scale=scale_sq)
        nc.gpsimd.partition_broadcast(out_bcast[0:chans, sl], r[0:1, :],
                                      channels=chans)


def build_program():
    nc = bacc.Bacc("TRN2", target_bir_lowering=False, debug=False, num_devices=8)

    def mm3(out_ap, whi, wlo, xhi, xlo, first, last):
        nc.tensor.matmul(out_ap, whi, xhi, start=first, stop=False,
                         perf_mode=DR)
        nc.tensor.matmul(out_ap, wlo, xhi, start=False, stop=False,
                         perf_mode=DR)
        nc.tensor.matmul(out_ap, whi, xlo, start=False, stop=last,
                         perf_mode=DR)

    # fp8 inputs (host pre-scaled; x2/gu hi/lo splits happen on device)
    xT_hi = nc.dram_tensor("xT_hi", [4, P, DT * 512], FP8, kind="ExternalInput")
    xq_hi = nc.dram_tensor("xq_hi", [P, DT * T], FP8, kind="ExternalInput")
    xTq32 = nc.dram_tensor("xTq32", [P, DT * T], F32, kind="ExternalInput")
    qaw_hi = nc.dram_tensor("qaw_hi", [2, P, DT * 512], FP8, kind="ExternalInput")
    qbn_hi = nc.dram_tensor("qbn_hi", [H, P, QLT * P], FP8, kind="ExternalInput")
    qbr_hi = nc.dram_tensor("qbr_hi", [H // 2, P, QLT * P], FP8,
                            kind="ExternalInput")
    kva_hi = nc.dram_tensor("kva_hi", [P, DT * (KVL + ROPE)], FP8,
                            kind="ExternalInput")
    kbk_hi = nc.dram_tensor("kbk_hi", [H, P, KVT * P], FP8, kind="ExternalInput")
    kbv_hi = nc.dram_tensor("kbv_hi", [4, P, KVT * 512], FP8,
                            kind="ExternalInput")
    ow_hi = nc.dram_tensor("ow_hi", [DT, P, H * P], FP8, kind="ExternalInput")
    gw_hi = nc.dram_tensor("gw_hi", [FFT // 2, P, DT * 256], FP8,
                           kind="ExternalInput")
    gw_lo = nc.dram_tensor("gw_lo", [FFT // 2, P, DT * 256], FP8,
                           kind="ExternalInput")
    uw_hi = nc.dram_tensor("uw_hi", [FFT // 2, P, DT * 256], FP8,
                           kind="ExternalInput")
    uw_lo = nc.dram_tensor("uw_lo", [FFT // 2, P, DT * 256], FP8,
                           kind="ExternalInput")
    dw_hi = nc.dram_tensor("dw_hi", [4, FFT // 8, P, 8 * 512], FP8,
                           kind="ExternalInput")
    dw_lo = nc.dram_tensor("dw_lo", [4, FFT // 8, P, 8 * 512], FP8,
                           kind="ExternalInput")
    cosKT = nc.dram_tensor("cosKT", [ROPE, S], BF16, kind="ExternalInput")
    sinKT = nc.dram_tensor("sinKT", [ROPE, S], BF16, kind="ExternalInput")
    cosQ2 = nc.dram_tensor("cosQ2", [P, T], BF16, kind="ExternalInput")
    sinQ2 = nc.dram_tensor("sinQ2", [P, T], BF16, kind="ExternalInput")
    p2t = nc.dram_tensor("p2t", [P, P], BF16, kind="ExternalInput")
    maskM = nc.dram_tensor("maskM", [P, NCH * P], BF16, kind="ExternalInput")
    ones_in = nc.dram_tensor("ones_in", [P, 1], BF16, kind="ExternalInput")
    ones8_in = nc.dram_tensor("ones8_in", [P, 32], FP8, kind="ExternalInput")
    out = nc.dram_tensor("out", [D, T], F32, kind="ExternalOutput")

    with TileContext(nc) as tc:
        with tc.tile_pool(name="pp", bufs=1) as pp, \
             tc.tile_pool(name="dram", bufs=1, space="DRAM") as dp:

            # ---------------- persistent constants & cross-stage tensors
            ones_sb = pp.tile([P, 1], BF16, tag="ones")
            ones8_sb = pp.tile([P, 32], FP8, tag="ones8")
            nc.sync.dma_start(ones_sb[:], ones_in[:, :])
            nc.sync.dma_start(ones8_sb[:], ones8_in[:, :])

            p2t_sb = pp.tile([P, P], BF16, tag="p2t")
            mask_sb = pp.tile([P, NCH * P], BF16, tag="mask")
            at_hi = pp.tile([P, H * T], FP8, tag="at_hi")

            kfin8_dram = dp.tile([P, S], FP8, tag="kfin8")
            khT8_dram = dp.tile([H, P, 4, 512], FP8, tag="khT8")

            vpool_cm = tc.tile_pool(name="vp", bufs=1)
            vp = vpool_cm.__enter__()
            v8_sb = vp.tile([P, NCH * H * VH], FP8, tag="v8")
            qj8 = vp.tile([P, 2 * H * T], FP8, tag="qj8")
            # zero the rope slot's pad rows once (killed rows for the joint
            # score matmul; rows 0:64 are overwritten per head)
            nc.vector.memset(qj8[64:128, H * T:2 * H * T], 0.0)

            # stage B persistents opened early so B's inputs can prefetch
            # during stage A (their DMAs are issued inside A's n-loop)
            bq_cm = tc.tile_pool(name="bq_pp", bufs=1)
            bqp = bq_cm.__enter__()
            qa_hi = bqp.tile([P, QLT * T], FP8, tag="qa_hi")
            cosQ_sb = bqp.tile([P, T], BF16, tag="cosQ")
            sinQ_sb = bqp.tile([P, T], BF16, tag="sinQ")
            xqh_all = bqp.tile([P, DT * T], FP8, tag="xqh")

            # ============================ stage A: kv path over full sequence
            with tc.tile_pool(name="a_pp", bufs=1) as app, \
                 tc.tile_pool(name="a_st", bufs=2) as ast, \
                 tc.tile_pool(name="a_sc", bufs=3) as asc, \
                 tc.tile_pool(name="a_sq", bufs=1) as asq:

                kwh = app.tile([P, DT * (KVL + ROPE)], FP8, tag="kwh")
                ckv_hi = app.tile([P, KVT * S], FP8, tag="ckv_hi")
                rx_b = app.tile([ROPE, S], F32, tag="rx_b")
                kr_bf = app.tile([ROPE, S], BF16, tag="krbf")
                kfin8_sb = app.tile([ROPE, S], FP8, tag="kfin8s")
                cosK_sb = app.tile([ROPE, S], BF16, tag="cosK")
                sinK_sb = app.tile([ROPE, S], BF16, tag="sinK")

                a1b = tc.tile_pool(name="a_p1", bufs=2, space="PSUM")
                ap1 = a1b.__enter__()
                a1 = tc.tile_pool(name="a_ps", bufs=1, space="PSUM")
                aps = a1.__enter__()

                KW = KVL + ROPE
                kwh_r = kwh[:].rearrange("p (d c) -> p d c", d=DT)

                for n in range(S // 512):
                    pts = [aps.tile([P, 512], F32, name=f"kva{m}", tag=f"kva{m}")
                           for m in range(5)]
                    psx = ap1.tile([1, 512], F32, tag="ps1")
                    xth = ast.tile([P, DT * 512], FP8, tag="xth", bufs=2)
                    if n == 0:
                        # fine-grained startup: first compute unit's inputs
                        # arrive first (kw pair dp, then xt pair dp)
                        for dpi in range(DP):
                            nc.sync.dma_start(
                                kwh[:, dpi * 2 * KW:(dpi + 1) * 2 * KW],
                                kva_hi[:, dpi * 2 * KW:(dpi + 1) * 2 * KW])
                            nc.sync.dma_start(
                                xth[:, dpi * 1024:(dpi + 1) * 1024],
                                xT_hi[n, :, dpi * 1024:(dpi + 1) * 1024])
                            if dpi == 1:
                                nc.sync.dma_start(cosK_sb[:], cosKT[:, :])
                            if dpi == 3:
                                nc.sync.dma_start(sinK_sb[:], sinKT[:, :])
                    else:
                        nc.sync.dma_start(xth[:], xT_hi[n, :, :])
                    if n == 1:
                        nc.sync.dma_start(p2t_sb[:], p2t[:, :])
                        nc.sync.dma_start(mask_sb[:], maskM[:, :])
                    if n == 2:
                        nc.sync.dma_start(xqh_all[:], xq_hi[:, :])
                        nc.sync.dma_start(cosQ_sb[:], cosQ2[:, :])
                        nc.sync.dma_start(sinQ_sb[:], sinQ2[:, :])


                    xth_r = xth[:].rearrange("p (d c) -> p d c", d=DT)
                    # x^2 stats: squares spread over DVE/Act, pairwise adds
                    # on DVE/Pool, then 8 accumulating ones-matmuls on the PE
                    for i in range(8):
                        xa = xth[:, (2 * i) * 512:(2 * i + 1) * 512]
                        xb = xth[:, (2 * i + 1) * 512:(2 * i + 2) * 512]
                        sqa = asc.tile([P, 512], BF16, tag="sqxa", bufs=4)
                        sqb = asc.tile([P, 512], BF16, tag="sqxb", bufs=4)
                        nc.vector.tensor_tensor(sqa[:], xa, xa, ALU.mult)
                        nc.scalar.square(sqb[:], xb)
                        eng = nc.gpsimd if i % 2 == 0 else nc.vector
                        eng.tensor_tensor(sqa[:], sqa[:], sqb[:], ALU.add)
                        nc.tensor.matmul(psx[0:1, :], ones_sb[:], sqa[:],
                                         start=(i == 0), stop=(i == 7))
                    for dpi in range(DP):
                        xh = xth_r[:, 2 * dpi:2 * dpi + 2, :]
                        first = dpi == 0
                        last = dpi == DP - 1
                        for m in range(5):
                            mp = P if m < 4 else ROPE
                            c0, c1 = m * P, m * P + mp
                            nc.tensor.matmul(
                                pts[m][:mp, :],
                                kwh_r[:, 2 * dpi:2 * dpi + 2, c0:c1],
                                xh, start=first, stop=last, perf_mode=DR)
                    _rstat(nc, asq, psx, 1.0 / (D * SX * SX),
                           (SKN / (SW * SX)) ** 2,
                           rx_b[:, n * 512:(n + 1) * 512], ROPE, 512)
                    # ckv chunk: evict, stats, normalize, hi/lo split inline
                    ckv_cn = asc.tile([P, KVT * 512], BF16, tag="ckvn",
                                      bufs=2)
                    for m in range(4):
                        nc.scalar.activation(ckv_cn[:, m * 512:(m + 1) * 512],
                                             pts[m][:], AF.Copy,
                                             scale=SC / (SW * SX))
                    nc.scalar.activation(kr_bf[:, n * 512:(n + 1) * 512],
                                         pts[4][0:ROPE, :], AF.Copy)
                    pskv = ap1.tile([1, 512], F32, tag="ps1")
                    for kvt in range(KVT):
                        sq2 = asc.tile([P, 512], BF16, tag="sq")
                        eng = nc.gpsimd if kvt % 2 == 0 else nc.vector
                        eng.tensor_tensor(
                            sq2[:], ckv_cn[:, kvt * 512:(kvt + 1) * 512],
                            ckv_cn[:, kvt * 512:(kvt + 1) * 512], ALU.mult)
                        nc.tensor.matmul(pskv[0:1, :], ones_sb[:], sq2[:],
                                         start=(kvt == 0), stop=(kvt == KVT - 1))
                    rkv_n = asc.tile([P, 512], F32, tag="rkvn", bufs=2)
                    _rstat(nc, asq, pskv, 1.0 / (KVL * SC * SC), 1.0,
                           rkv_n[:], P, 512)
                    for kvt in range(KVT):
                        cs = slice(kvt * 512, (kvt + 1) * 512)
                        sl = slice(kvt * S + n * 512, kvt * S + n * 512 + 512)
                        nc.vector.tensor_tensor(ckv_hi[:, sl], ckv_cn[:, cs],
                                                rkv_n[:], ALU.mult)

                a1.__exit__(None, None, None)
                a1b.__exit__(None, None, None)

                # k_rope rot + cos/sin + rx scale -> kfin8
                a3 = tc.tile_pool(name="a_p3", bufs=2, space="PSUM")
                ap3 = a3.__enter__()
                for n in range(S // 512):
                    pr = ap3.tile([P, 512], F32, tag="rot")
                    nc.tensor.matmul(pr[0:ROPE, :], p2t_sb[0:ROPE, 0:ROPE],
                                     kr_bf[:, n * 512:(n + 1) * 512],
                                     start=True, stop=True)
                    m1t = asc.tile([ROPE, 512], F32, tag="km1", bufs=2)
                    nc.vector.tensor_tensor(m1t[:],
                                            kr_bf[:, n * 512:(n + 1) * 512],
                                            cosK_sb[:, n * 512:(n + 1) * 512],
                                            ALU.mult)
                    t2 = asc.tile([ROPE, 512], F32, tag="kt2", bufs=2)
                    nc.vector.tensor_tensor(t2[:], pr[0:ROPE, :],
                                            sinK_sb[:, n * 512:(n + 1) * 512],
                                            ALU.mult)
                    nc.vector.tensor_tensor(t2[:], t2[:], m1t[:], ALU.add)
                    nc.vector.tensor_tensor(kfin8_sb[:, n * 512:(n + 1) * 512],
                                            t2[:],
                                            rx_b[:, n * 512:(n + 1) * 512],
                                            ALU.mult)
                nc.sync.dma_start(kfin8_dram[0:ROPE, :], kfin8_sb[:])
                z64 = asc.tile([ROPE, S], FP8, tag="z64", bufs=1)
                nc.vector.memset(z64[:], 0.0)
                nc.sync.dma_start(kfin8_dram[ROPE:P, :], z64[:])

                ckvh_r = ckv_hi[:].rearrange("p (k s) -> p k s", k=KVT)

                # kv_b k-half: khs8[h] -> DRAM
                for h in range(H):
                    kbkh = ast.tile([P, KVT * P], FP8, tag="kbkh")
                    nc.sync.dma_start(kbkh[:], kbk_hi[h, :, :])
                    kbkh_r = kbkh[:].rearrange("p (k c) -> p k c", k=KVT)
                    for n in range(S // 512):
                        pt = ap3.tile([P, 512], F32, tag="kb", bufs=3)
                        for kp in range(KVT // 2):
                            nc.tensor.matmul(
                                pt[:],
                                kbkh_r[:, 2 * kp:2 * kp + 2, :],
                                ckvh_r[:, 2 * kp:2 * kp + 2,
                                       n * 512:(n + 1) * 512],
                                start=kp == 0, stop=kp == KVT // 2 - 1,
                                perf_mode=DR)
                        khs = asc.tile([P, 512], FP8, tag="khs")
                        if (h + n) % 2 == 0:
                            nc.scalar.activation(khs[:], pt[:], AF.Copy,
                                                 scale=SKN / (SW * SC))
                        else:
                            nc.vector.tensor_scalar_mul(khs[:], pt[:],
                                                        SKN / (SW * SC))
                        nc.sync.dma_start(khT8_dram[h, :, n, :], khs[:])

                # kv_b v-half: v8 rows=token, cols=(h,vh) -> SBUF resident
                for n4 in range(4):
                    kbvh = ast.tile([P, KVT * 512], FP8, tag="kbvh", bufs=2)
                    nc.sync.dma_start(kbvh[:], kbv_hi[n4, :, :])
                    kbvh_r = kbvh[:].rearrange("p (k c) -> p k c", k=KVT)
                    for tt in range(NCH):
                        pt = ap3.tile([P, 512], F32, tag="vb", bufs=3)
                        for kp in range(KVT // 2):
                            nc.tensor.matmul(
                                pt[:],
                                ckvh_r[:, 2 * kp:2 * kp + 2,
                                       tt * P:(tt + 1) * P],
                                kbvh_r[:, 2 * kp:2 * kp + 2, :],
                                start=kp == 0, stop=kp == KVT // 2 - 1,
                                perf_mode=DR)
                        v8_ap = v8_sb[:, tt * H * VH + n4 * 512:
                                      tt * H * VH + (n4 + 1) * 512]
                        if (n4 + tt) % 2 == 0:
                            nc.scalar.activation(v8_ap, pt[:], AF.Copy,
                                                 scale=SV / (SW * SC))
                        else:
                            nc.vector.tensor_scalar_mul(v8_ap, pt[:],
                                                        SV / (SW * SC))
                a3.__exit__(None, None, None)

            # ============================ stage B: q_a (core's T tokens)
            with tc.tile_pool(name="b_st", bufs=2) as bst, \
                 tc.tile_pool(name="b_sc", bufs=3) as bsc:
                xqh_r = xqh_all[:].rearrange("p (d c) -> p d c", d=DT)
                qa_c = bst.tile([P, QLT * T], BF16, tag="qa_c", bufs=1)
                rq_b = bst.tile([P, T], F32, tag="rq_b", bufs=1)

                b1 = tc.tile_pool(name="b_ps", bufs=1, space="PSUM")
                bps = b1.__enter__()
                psq = bps.tile([1, T], F32, tag="psq")
                for half in range(2):
                    pts = [bps.tile([P, T], F32, name=f"qa{m}", tag=f"qa{m}")
                           for m in range(4)]
                    qwh = bst.tile([P, DT * 512], FP8, tag="qwh")
                    nc.sync.dma_start(qwh[:], qaw_hi[half, :, :])
                    qwh_r = qwh[:].rearrange("p (d c) -> p d c", d=DT)
                    for dpi in range(DP):
                        for m in range(4):
                            c0, c1 = m * P, (m + 1) * P
                            nc.tensor.matmul(
                                pts[m][:],
                                qwh_r[:, 2 * dpi:2 * dpi + 2, c0:c1],
                                xqh_r[:, 2 * dpi:2 * dpi + 2, :],
                                start=(dpi == 0), stop=(dpi == DP - 1),
                                perf_mode=DR)
                    for m in range(4):
                        mi = half * 4 + m
                        sl = slice(mi * T, (mi + 1) * T)
                        nc.scalar.activation(qa_c[:, sl], pts[m][:], AF.Copy,
                                             scale=SQ / (SW * SX))
                        sqb = bsc.tile([P, T], BF16, tag="sqb")
                        eng = nc.gpsimd if m % 2 == 0 else nc.vector
                        eng.tensor_tensor(sqb[:], qa_c[:, sl],
                                          qa_c[:, sl], ALU.mult)
                        nc.tensor.matmul(psq[0:1, :], ones_sb[:], sqb[:],
                                         start=(mi == 0), stop=(mi == QLT - 1))
                # rq carries an extra 8x so qa_hi lands in fp8 normal range;
                # compensated in the exp scale (SQF)
                _rstat(nc, bqp, psq, 1.0 / (QL * SQ * SQ),
                       (8.0 * SM * SQN / (SW * SQ)) ** 2, rq_b, P, T)
                # fold the per-token q normalization into qa_hi: the q_b
                # matmul is linear, so scaling its input columns by rq is
                # identical to scaling its output columns
                for mi in range(QLT):
                    sl = slice(mi * T, (mi + 1) * T)
                    eng = nc.gpsimd if mi % 4 == 3 else nc.vector
                    eng.tensor_tensor(qa_hi[:, sl], qa_c[:, sl], rq_b[:],
                                      ALU.mult)
                b1.__exit__(None, None, None)

            # ============================ fused stage C: q_b + attention
            qah_r = qa_hi[:].rearrange("p (k c) -> p k c", k=QLT)
            qj_r = qj8[:].rearrange("p (t x) -> p t x", t=2)
            v8_r = v8_sb[:].rearrange("p (k c) -> p k c", k=NCH)
            ones8_r = ones8_sb[:].rearrange("p (t o) -> p t o", t=2)[:, :, 0:1]

            with tc.tile_pool(name="c_st", bufs=3) as cst, \
                 tc.tile_pool(name="c_w", bufs=2) as cw, \
                 tc.tile_pool(name="c_pr", bufs=3) as cpr, \
                 tc.tile_pool(name="c_ps", bufs=1, space="PSUM") as cps, \
                 tc.tile_pool(name="c_pt", bufs=2, space="PSUM") as cpt, \
                 tc.tile_pool(name="c_pq", bufs=2, space="PSUM") as cpq:

                def emit_qb(h):
                    """q_b nope (and rope when h is even) for head h -> qj8."""
                    nbh = cw.tile([P, QLT * P], FP8, tag="nbh")
                    nc.sync.dma_start(nbh[:], qbn_hi[h, :, :])
                    nbh_r = nbh[:].rearrange("p (k c) -> p k c", k=QLT)
                    pt = cpq.tile([P, T], F32, tag="qb")
                    for mp in range(QLT // 2):
                        nc.tensor.matmul(
                            pt[:],
                            nbh_r[:, 2 * mp:2 * mp + 2, :],
                            qah_r[:, 2 * mp:2 * mp + 2, :],
                            start=mp == 0, stop=mp == QLT // 2 - 1,
                            perf_mode=DR)
                    nc.vector.tensor_copy(qj8[:, h * T:(h + 1) * T], pt[:])
                    if h % 2 == 0:
                        g = h // 2
                        rbh = cw.tile([P, QLT * P], FP8, tag="rbh")
                        nc.sync.dma_start(rbh[:], qbr_hi[g, :, :])
                        rbh_r = rbh[:].rearrange("p (k c) -> p k c", k=QLT)
                        ptr = cpq.tile([P, T], F32, tag="qb", name="qbr")
                        for mp in range(QLT // 2):
                            nc.tensor.matmul(
                                ptr[:],
                                rbh_r[:, 2 * mp:2 * mp + 2, :],
                                qah_r[:, 2 * mp:2 * mp + 2, :],
                                start=mp == 0, stop=mp == QLT // 2 - 1,
                                perf_mode=DR)
                        qrb = cpr.tile([P, T], BF16, tag="qrb")
                        nc.vector.tensor_copy(qrb[:], ptr[:])
                        mm1 = cpr.tile([P, T], F32, tag="mm1")
                        nc.vector.tensor_tensor(mm1[:], ptr[:], cosQ_sb[:],
                                                ALU.mult)
                        prot = cpq.tile([P, T], F32, tag="qb", name="rot")
                        nc.tensor.matmul(prot[:], p2t_sb[:], qrb[:],
                                         start=True, stop=True)
                        t2 = cpr.tile([P, T], F32, tag="bt2")
                        nc.vector.tensor_tensor(t2[:], prot[:], sinQ_sb[:],
                                                ALU.mult)
                        qrf = cpr.tile([P, T], FP8, tag="qrf")
                        nc.vector.tensor_tensor(qrf[:], t2[:], mm1[:],
                                                ALU.add)
                        base = H * T
                        nc.sync.dma_start(
                            qj8[0:ROPE, base + 2 * g * T:base + (2 * g + 1) * T],
                            qrf[0:ROPE, :])
                        nc.sync.dma_start(
                            qj8[0:ROPE,
                                base + (2 * g + 1) * T:base + (2 * g + 2) * T],
                            qrf[ROPE:P, :])

                emit_qb(0)
                emit_qb(1)
                for h in range(H):
                    kj = cst.tile([P, 2 * S], FP8, tag="kj")
                    nc.sync.dma_start(kj[:, 0:S], khT8_dram[h, :, :, :])
                    nc.sync.dma_start(kj[:, S:2 * S], kfin8_dram[:, :])
                    kj_r = kj[:].rearrange("p (t s) -> p t s", t=2)
                    pa = cps.tile([P, T], F32, tag="pa")
                    ps = cps.tile([1, T], F32, tag="ps")

                    for m in range(NCH // 2):
                        j0 = m // 2
                        q0 = j0 * P
                        w = T - q0
                        pt2 = cpt.tile([P, 2 * T], F32, tag="pt2")
                        pt2_r = pt2[:].rearrange("p (t x) -> p t x", t=2)
                        for t in range(2):
                            kt = 2 * m + t
                            nc.tensor.matmul(
                                pt2[:, t * T:t * T + w],
                                kj_r[:, :, kt * P:(kt + 1) * P],
                                qj_r[:, :, h * T + q0:h * T + T],
                                start=True, stop=True, perf_mode=DR)
                        probs2 = cpr.tile([P, 2 * T], FP8, tag="probs2",
                                          name="probs2")
                        probs2_r = probs2[:].rearrange("p (t x) -> p t x", t=2)
                        nc.scalar.activation(probs2_r[:, :, 0:w],
                                             pt2_r[:, :, 0:w], AF.Exp,
                                             scale=1.0 / (SKN * SQN * 8.0))
                        mask2 = mask_sb[:, (2 * m) * P:(2 * m + 2) * P
                                        ].rearrange("p (t x) -> p t x", t=2)
                        eng = nc.vector if m % 2 == 0 else nc.gpsimd
                        eng.tensor_tensor(probs2_r[:, :, 0:P],
                                          probs2_r[:, :, 0:P],
                                          mask2, ALU.mult)
                        nc.tensor.matmul(ps[0:1, q0:T], ones8_r,
                                         probs2_r[:, :, 0:w],
                                         start=(m == 0), stop=(m == NCH // 2 - 1),
                                         perf_mode=DR)
                        for j in range(j0, NQ):
                            c0 = j * P - q0
                            nc.tensor.matmul(
                                pa[:, j * P:(j + 1) * P],
                                v8_r[:, 2 * m:2 * m + 2, h * VH:(h + 1) * VH],
                                probs2_r[:, :, c0:c0 + P],
                                start=(m == 0 and j == 0),
                                stop=(m == 2 * j + 1), perf_mode=DR)

                    if h + 2 < H:
                        emit_qb(h + 2)
                    rs = cpr.tile([1, T], F32, tag="rs")
                    nc.vector.reciprocal(rs[0:1, :], ps[0:1, :])
                    rsb = cpr.tile([P, T], F32, tag="rsb")
                    nc.gpsimd.partition_broadcast(rsb[:], rs[0:1, :], channels=P)
                    nc.vector.tensor_tensor(at_hi[:, h * T:(h + 1) * T],
                                            pa[:], rsb[:], ALU.mult)

            bq_cm.__exit__(None, None, None)
            vpool_cm.__exit__(None, None, None)

            # ===== persistents for D..F (opened after the A..C pools close)
            fpp_cm = tc.tile_pool(name="f_pp", bufs=1)
            fpp = fpp_cm.__enter__()
            x2T = fpp.tile([P, DT * T], F32, tag="x2T")
            x2hi = fpp.tile([P, DT * T], FP8, tag="x2hi")
            x2lo = fpp.tile([P, DT * T], FP8, tag="x2lo")
            gu_hi = fpp.tile([P, FFT * T], FP8, tag="gu_hi")
            gu_lo = fpp.tile([P, FFT * T], FP8, tag="gu_lo")

            # ============================ stage D: o_proj + residual + norm
            ath_r = at_hi[:].rearrange("p (k c) -> p k c", k=H)

            est_cm = tc.tile_pool(name="e_st", bufs=2)
            est = est_cm.__enter__()
            epre = {}
            with tc.tile_pool(name="d_st", bufs=1) as dst, \
                 tc.tile_pool(name="d_w", bufs=2) as dw, \
                 tc.tile_pool(name="d_sc", bufs=3) as dsc, \
                 tc.tile_pool(name="d_ps", bufs=3, space="PSUM") as dps:
                # prefetch stage E's first gate/up block
                for nm, src in [("gwh", gw_hi), ("gwl", gw_lo),
                                ("uwh", uw_hi), ("uwl", uw_lo)]:
                    tile = est.tile([P, DT * 256], FP8, tag=nm, name=nm + "0")
                    nc.sync.dma_start(tile[:], src[0, :, :])
                    epre[nm] = tile

                dp1 = tc.tile_pool(name="d_p1", bufs=1, space="PSUM")
                dps1 = dp1.__enter__()
                ps2 = dps1.tile([1, T], F32, tag="ps2")
                for dt in range(DT):
                    odh = dw.tile([P, H * P], FP8, tag="odh")
                    nc.sync.dma_start(odh[:], ow_hi[dt, :, :])
                    odh_r = odh[:].rearrange("p (k c) -> p k c", k=H)
                    po = dps.tile([P, T], F32, tag="po")
                    for hp in range(H // 2):
                        nc.tensor.matmul(
                            po[:],
                            odh_r[:, 2 * hp:2 * hp + 2, :],
                            ath_r[:, 2 * hp:2 * hp + 2, :],
                            start=hp == 0, stop=hp == H // 2 - 1,
                            perf_mode=DR)
                    xq32_t = dsc.tile([P, T], F32, tag="xq32")
                    nc.sync.dma_start(xq32_t[:], xTq32[:, dt * T:(dt + 1) * T])
                    nc.vector.scalar_tensor_tensor(
                        x2T[:, dt * T:(dt + 1) * T], po[:], 1.0 / (SW * SV),
                        xq32_t[:], ALU.mult, ALU.add)
                    sqd = dsc.tile([P, T], BF16, tag="sqd")
                    if dt % 2 == 0:
                        nc.gpsimd.tensor_tensor(
                            sqd[:], x2T[:, dt * T:(dt + 1) * T],
                            x2T[:, dt * T:(dt + 1) * T], ALU.mult)
                    else:
                        nc.scalar.square(sqd[:], x2T[:, dt * T:(dt + 1) * T])
                    nc.tensor.matmul(ps2[0:1, :], ones_sb[:], sqd[:],
                                     start=(dt == 0), stop=(dt == DT - 1))
                r2b = dst.tile([P, T], F32, tag="r2b")
                _rstat(nc, dst, ps2, 1.0 / D, SX * SX, r2b, P, T)
                dp1.__exit__(None, None, None)
                for dt in range(DT):
                    sl = slice(dt * T, (dt + 1) * T)
                    x2c = dsc.tile([P, T], BF16, tag="x2c")
                    eng = nc.gpsimd if dt % 4 == 3 else nc.vector
                    eng.tensor_tensor(x2c[:], x2T[:, sl], r2b[:], ALU.mult)
                    nc.scalar.activation(x2hi[:, sl], x2c[:], AF.Copy)
                    eng2 = nc.gpsimd if dt % 4 == 1 else nc.vector
                    eng2.tensor_tensor(x2lo[:, sl], x2c[:], x2hi[:, sl],
                                       ALU.subtract)

            # ============================ stage E: FFN gate/up -> gu hi/lo
            x2h_r = x2hi[:].rearrange("p (d c) -> p d c", d=DT)
            x2l_r = x2lo[:].rearrange("p (d c) -> p d c", d=DT)

            fst_cm = tc.tile_pool(name="f_st", bufs=2)
            fst = fst_cm.__enter__()
            pre = {}
            with tc.tile_pool(name="e_sc", bufs=3) as esc, \
                 tc.tile_pool(name="e_ps", bufs=3, space="PSUM") as eps:
                # prefetch stage F's first down_w blocks
                for k in range(2):
                    dwbh = fst.tile([P, 8 * 512], FP8, tag=f"dwh{k}",
                                    name=f"dwh{k}")
                    dwbl = fst.tile([P, 8 * 512], FP8, tag=f"dwl{k}",
                                    name=f"dwl{k}")
                    nc.sync.dma_start(dwbh[:], dw_hi[k, 0, :, :])
                    nc.sync.dma_start(dwbl[:], dw_lo[k, 0, :, :])
                    pre[f"dwh{k}"] = dwbh
                    pre[f"dwl{k}"] = dwbl
                for fb in range(FFT // 2):
                    if fb == 0:
                        gwh, gwl = epre["gwh"], epre["gwl"]
                        uwh, uwl = epre["uwh"], epre["uwl"]
                    else:
                        gwh = est.tile([P, DT * 256], FP8, tag="gwh")
                        gwl = est.tile([P, DT * 256], FP8, tag="gwl")
                        uwh = est.tile([P, DT * 256], FP8, tag="uwh")
                        uwl = est.tile([P, DT * 256], FP8, tag="uwl")
                        nc.sync.dma_start(gwh[:], gw_hi[fb, :, :])
                        nc.sync.dma_start(gwl[:], gw_lo[fb, :, :])
                        nc.sync.dma_start(uwh[:], uw_hi[fb, :, :])
                        nc.sync.dma_start(uwl[:], uw_lo[fb, :, :])
                    gwh_r = gwh[:].rearrange("p (d c) -> p d c", d=DT)
                    gwl_r = gwl[:].rearrange("p (d c) -> p d c", d=DT)
                    uwh_r = uwh[:].rearrange("p (d c) -> p d c", d=DT)
                    uwl_r = uwl[:].rearrange("p (d c) -> p d c", d=DT)
                    for i in range(2):
                        f = fb * 2 + i
                        pg = eps.tile([P, T], F32, tag="pg")
                        pu = eps.tile([P, T], F32, tag="pu")
                        for dpi in range(DP):
                            c0, c1 = i * P, (i + 1) * P
                            mm3(pg[:],
                                gwh_r[:, 2 * dpi:2 * dpi + 2, c0:c1],
                                gwl_r[:, 2 * dpi:2 * dpi + 2, c0:c1],
                                x2h_r[:, 2 * dpi:2 * dpi + 2, :],
                                x2l_r[:, 2 * dpi:2 * dpi + 2, :],
                                dpi == 0, dpi == DP - 1)
                        for dpi in range(DP):
                            c0, c1 = i * P, (i + 1) * P
                            mm3(pu[:],
                                uwh_r[:, 2 * dpi:2 * dpi + 2, c0:c1],
                                uwl_r[:, 2 * dpi:2 * dpi + 2, c0:c1],
                                x2h_r[:, 2 * dpi:2 * dpi + 2, :],
                                x2l_r[:, 2 * dpi:2 * dpi + 2, :],
                                dpi == 0, dpi == DP - 1)
                        gs = esc.tile([P, T], BF16, tag="gs")
                        nc.scalar.activation(gs[:], pg[:], AF.Silu,
                                             scale=1.0 / (SW * SX))
                        guc = esc.tile([P, T], BF16, tag="guc")
                        nc.vector.scalar_tensor_tensor(
                            guc[:], gs[:], SG / (SW * SX), pu[:],
                            ALU.mult, ALU.mult)
                        sl = slice(f * T, (f + 1) * T)
                        if f % 2 == 0:
                            nc.scalar.activation(gu_hi[:, sl], guc[:], AF.Copy)
                        else:
                            nc.vector.tensor_copy(gu_hi[:, sl], guc[:])
                        eng = nc.gpsimd if f % 2 == 0 else nc.vector
                        eng.tensor_tensor(gu_lo[:, sl], guc[:],
                                          gu_hi[:, sl], ALU.subtract)

            # ============================ stage F: down proj + residual
            guh_r = gu_hi[:].rearrange("p (k c) -> p k c", k=FFT)
            gul_r = gu_lo[:].rearrange("p (k c) -> p k c", k=FFT)
            with tc.tile_pool(name="f_sc", bufs=2) as fsc, \
                 tc.tile_pool(name="f_ps", bufs=1, space="PSUM") as fps:
                for np2 in range(D // 1024):
                    pds = [fps.tile([P, T], F32, name=f"pd{i}", tag=f"pd{i}")
                           for i in range(2 * NQ)]
                    for fb8 in range(FFT // 8):
                        if np2 == 0 and fb8 == 0:
                            dwbhs = [pre["dwh0"], pre["dwh1"]]
                            dwbls = [pre["dwl0"], pre["dwl1"]]
                        else:
                            dwbhs, dwbls = [], []
                            for k in range(2):
                                dwbh = fst.tile([P, 8 * 512], FP8,
                                                tag=f"dwh{k}", name=f"dwh{k}")
                                dwbl = fst.tile([P, 8 * 512], FP8,
                                                tag=f"dwl{k}", name=f"dwl{k}")
                                nc.sync.dma_start(
                                    dwbh[:], dw_hi[np2 * 2 + k, fb8, :, :])
                                nc.sync.dma_start(
                                    dwbl[:], dw_lo[np2 * 2 + k, fb8, :, :])
                                dwbhs.append(dwbh)
                                dwbls.append(dwbl)
                        dwh_rs = [d[:].rearrange("p (f c) -> p f c", f=8)
                                  for d in dwbhs]
                        dwl_rs = [d[:].rearrange("p (f c) -> p f c", f=8)
                                  for d in dwbls]
                        if fb8 < FFT // 8 - 2:
                            for fip in range(4):
                                for k in range(2):
                                    for i in range(4):
                                        fpair = fb8 * 8 + 2 * fip
                                        mm3(pds[k * NQ + i][:],
                                            dwh_rs[k][:, 2 * fip:2 * fip + 2,
                                                      i * P:(i + 1) * P],
                                            dwl_rs[k][:, 2 * fip:2 * fip + 2,
                                                      i * P:(i + 1) * P],
                                            guh_r[:, fpair:fpair + 2, :],
                                            gul_r[:, fpair:fpair + 2, :],
                                            fb8 == 0 and fip == 0, False)
                        elif fb8 == FFT // 8 - 2:
                            blk_saved = (dwh_rs, dwl_rs)
                        else:
                            pvh, pvl = blk_saved
                            for k in range(2):
                                for i in range(4):
                                    for bi, (bh, bl) in enumerate(
                                            [(pvh, pvl), (dwh_rs, dwl_rs)]):
                                        bb = fb8 - 1 + bi
                                        for fip in range(4):
                                            fpair = bb * 8 + 2 * fip
                                            mm3(pds[k * NQ + i][:],
                                                bh[k][:, 2 * fip:2 * fip + 2,
                                                      i * P:(i + 1) * P],
                                                bl[k][:, 2 * fip:2 * fip + 2,
                                                      i * P:(i + 1) * P],
                                                guh_r[:, fpair:fpair + 2, :],
                                                gul_r[:, fpair:fpair + 2, :],
                                                False, bi == 1 and fip == 3)
                                    dt = (np2 * 2 + k) * 4 + i
                                    _evict_out(nc, fsc, pds[k * NQ + i], x2T,
                                               out, dt)


            fst_cm.__exit__(None, None, None)
            est_cm.__exit__(None, None, None)
            fpp_cm.__exit__(None, None, None)

    nc.compile()
    return nc


def _evict_out(nc, pool, pd, x2T, out, dt):
    ot = pool.tile([P, T], F32, tag="ot")
    eng = nc.gpsimd if dt % 4 == 3 else nc.vector
    eng.scalar_tensor_tensor(ot[:], pd[:], 1.0 / (SW * SG),
                             x2T[:, dt * T:(dt + 1) * T], ALU.mult, ALU.add)
    nc.sync.dma_start(out[dt * P:(dt + 1) * P, :], ot[:])


_PROGRAM = None


def _get_program():
    global _PROGRAM
    if _PROGRAM is None:
        _PROGRAM = build_program()
    return _PROGRAM


# ------------------------------------------------------------------- host

F8NP = ml_dtypes.float8_e4m3


def _q8(a, s):
    """Scale f32 array by s, round to e4m3."""
    x = np.asarray(a, np.float32) * np.float32(s)
    return np.ascontiguousarray(x.astype(F8NP))


def _hilo(a, s):
    """Scale f32 array by s, split into e4m3 hi + residual lo."""
    x = np.asarray(a, np.float32) * np.float32(s)
    hi = x.astype(F8NP)
    lo = (x - hi.astype(np.float32)).astype(F8NP)
    return np.ascontiguousarray(hi), np.ascontiguousarray(lo)


def _f32(a):
    return np.ascontiguousarray(a.astype(np.float32))


def kernel(**inputs):
    nc = _get_program()
    in_maps = _build_in_maps(**inputs)
    res = run_bass_kernel_spmd(nc, in_maps, core_ids=list(range(8)))
    return _gather(res.results)


def _gather(results):
    outp = np.empty((B, S, D), np.float32)
    for core in range(8):
        b, p = core // 4, core % 4
        chunks = chunks_for_pos(p)
        o = results[core]["out"]          # [D, T] transposed
        for j, c in enumerate(chunks):
            outp[b, c * P:(c + 1) * P, :] = o[:, j * P:(j + 1) * P].T
    return outp


def _build_in_maps(hidden_states, attn_norm_scale, q_a_w, q_a_norm_scale, q_b_w,
                   kv_a_w, kv_a_norm_scale, kv_b_w, o_w, ffn_norm_scale,
                   gate_w, up_w, down_w, attention_mask):
    hidden_states = np.asarray(hidden_states, dtype=np.float32)

    # fold norm scales into adjacent weights (exact fp32 host ops)
    ans = np.asarray(attn_norm_scale, np.float32)[:, None]
    q_a_wf = np.asarray(q_a_w, np.float32) * ans
    kv_a_wf = np.asarray(kv_a_w, np.float32) * ans
    q_b_wf = np.asarray(q_b_w, np.float32) * np.asarray(q_a_norm_scale,
                                                        np.float32)[:, None]
    kv_b_wf = np.asarray(kv_b_w, np.float32) * np.asarray(kv_a_norm_scale,
                                                          np.float32)[:, None]
    fns = np.asarray(ffn_norm_scale, np.float32)[:, None]
    gate_wf = np.asarray(gate_w, np.float32) * fns
    up_wf = np.asarray(up_w, np.float32) * fns

    qb = q_b_wf.reshape(QL, H, NOPE + ROPE)
    q_b_nope = (qb[:, :, :NOPE].reshape(QLT, P, H, NOPE)
                .transpose(2, 1, 0, 3).reshape(H, P, QLT * P))
    q_b_rope = (qb[:, :, NOPE:].reshape(QLT, P, H // 2, 2 * ROPE)
                .transpose(2, 1, 0, 3).reshape(H // 2, P, QLT * P))
    kb = kv_b_wf.reshape(KVL, H, NOPE + VH)
    kv_b_k = (kb[:, :, :NOPE].reshape(KVT, P, H, NOPE)
              .transpose(2, 1, 0, 3).reshape(H, P, KVT * P))
    kv_b_v = (kb[:, :, NOPE:].reshape(KVL, H * VH)
              .reshape(KVT, P, 4, 512)
              .transpose(2, 1, 0, 3).reshape(4, P, KVT * 512))

    # rope tables (match reference fp32 math)
    inv_freq = (1.0 / (BASE ** (np.arange(0, ROPE, 2, dtype=np.float32) / ROPE))
                ).astype(np.float32)
    ang = np.arange(S, dtype=np.float32)[:, None] * inv_freq[None, :]
    emb = np.concatenate([ang, ang], axis=-1)          # [S, ROPE]
    cos_t = np.cos(emb).astype(np.float32)
    sin_t = np.sin(emb).astype(np.float32)

    # rotation matrix: rot(x) = concat(-x[32:], x[:32]) = P64 @ x
    d2 = ROPE // 2
    P64 = np.zeros((ROPE, ROPE), np.float32)
    for i in range(d2):
        P64[i, i + d2] = -1.0
        P64[i + d2, i] = 1.0
    P2 = np.zeros((P, P), np.float32)
    P2[:ROPE, :ROPE] = P64
    P2[ROPE:, ROPE:] = P64
    p2t = np.ascontiguousarray(P2.T.astype(ml_dtypes.bfloat16))

    ones_in = np.ones((P, 1), ml_dtypes.bfloat16)
    ones8_in = np.ones((P, 32), F8NP)

    qaw_hi = _q8(q_a_wf.reshape(DT, P, 2, 512)
                 .transpose(2, 1, 0, 3).reshape(2, P, DT * 512), SW)
    kva_hi = _q8(kv_a_wf.reshape(DT, P, KVL + ROPE)
                 .transpose(1, 0, 2).reshape(P, DT * (KVL + ROPE)), SW)
    qbn_hi = _q8(q_b_nope, SW)
    qbr_hi = _q8(q_b_rope, SW)
    kbk_hi = _q8(kv_b_k, SW)
    kbv_hi = _q8(kv_b_v, SW)
    ow_hi = _q8(np.asarray(o_w, np.float32).reshape(H, P, DT, P)
                .transpose(2, 1, 0, 3).reshape(DT, P, H * P), SW)
    gw_hi, gw_lo = _hilo(gate_wf.reshape(DT, P, FFT // 2, 256)
                         .transpose(2, 1, 0, 3).reshape(FFT // 2, P, DT * 256),
                         SW)
    uw_hi, uw_lo = _hilo(up_wf.reshape(DT, P, FFT // 2, 256)
                         .transpose(2, 1, 0, 3).reshape(FFT // 2, P, DT * 256),
                         SW)
    dw_hi, dw_lo = _hilo(np.asarray(down_w, np.float32)
                         .reshape(FFT // 8, 8, P, 4, 512)
                         .transpose(3, 0, 2, 1, 4).reshape(4, FFT // 8, P,
                                                           8 * 512), SW)

    shared = {
        "qaw_hi": qaw_hi,
        "qbn_hi": qbn_hi,
        "qbr_hi": qbr_hi,
        "kva_hi": kva_hi,
        "kbk_hi": kbk_hi,
        "kbv_hi": kbv_hi,
        "ow_hi": ow_hi,
        "gw_hi": gw_hi, "gw_lo": gw_lo,
        "uw_hi": uw_hi, "uw_lo": uw_lo,
        "dw_hi": dw_hi, "dw_lo": dw_lo,
        "cosKT": np.ascontiguousarray(cos_t.T.astype(ml_dtypes.bfloat16)),
        "sinKT": np.ascontiguousarray(sin_t.T.astype(ml_dtypes.bfloat16)),
        "p2t": p2t, "ones_in": ones_in, "ones8_in": ones8_in,
    }

    in_maps = []
    for core in range(8):
        b, p = core // 4, core % 4
        chunks = chunks_for_pos(p)
        xb = hidden_states[b]                      # [S, D]
        xT_f = xb.T.astype(np.float32)             # [D, S]
        xTb = (xT_f.reshape(DT, P, 4, 512)
               .transpose(2, 1, 0, 3).reshape(4, P, DT * 512))
        xT_hi = _q8(xTb, SX)
        pos = np.concatenate([np.arange(c * P, (c + 1) * P) for c in chunks])
        xTq_blk = (xT_f[:, pos].reshape(DT, P, T).transpose(1, 0, 2)
                   .reshape(P, DT * T))
        xq_hi = _q8(xTq_blk, SX)
        xTq32 = _f32(xTq_blk)
        cq = cos_t[pos].T                          # [ROPE, T]
        sq = sin_t[pos].T
        cosQ2 = np.ascontiguousarray(
            np.concatenate([cq, cq], axis=0).astype(ml_dtypes.bfloat16))
        sinQ2 = np.ascontiguousarray(
            np.concatenate([sq, sq], axis=0).astype(ml_dtypes.bfloat16))
        # masks: [16] = (slot j, kt' = kt-4j); 1 allowed, 0 masked
        mk = np.zeros((NCH, P, P), np.float32)
        for j in range(NQ):
            ij = chunks[j] - 4 * j
            for kk in range(4):
                if kk < ij:
                    mk[4 * j + kk] = 1.0
                elif kk == ij:
                    mk[4 * j + kk] = np.triu(np.ones((P, P), np.float32))
        mk_r = mk.transpose(1, 0, 2).reshape(P, NCH * P)
        in_maps.append({
            "xT_hi": xT_hi,
            "xq_hi": xq_hi, "xTq32": xTq32,
            "cosQ2": cosQ2, "sinQ2": sinQ2,
            "maskM": np.ascontiguousarray(mk_r.astype(ml_dtypes.bfloat16)),
            **shared,
        })
    return in_maps


def run_traced(inputs):
    import os
    os.environ["BASS_PERFETTO_PROFILE_ALL_CORES"] = "1"
    nc = _get_program()
    in_maps = _build_in_maps(**inputs)
    res = run_bass_kernel_spmd(nc, in_maps, core_ids=list(range(8)), trace=True,
                               trace_cores=list(range(8)))
    return res.exec_time_ns


if __name__ == "__main__":
    _get_program()
    print("program built OK")

